# revision 1
# baseline (speedup 1.0000x reference)
"""Causal self-attention (B=2, T=2048, EMB=1024, 16 heads) on 8 TRN2 NeuronCores.

Sharding: core c handles batch c//4 and heads [4*(c%4), 4*(c%4)+4).
 - Wqkv is split column-wise per head group (q part pre-scaled by 1/sqrt(hd)),
 - Wproj is split row-wise per head group,
 - each core emits a partial [2048, 1024] projection output,
 - host sums the 4 partials per batch and adds bproj (row-parallel unshard).

Device kernel (per core, SPMD):
 - host supplies x^T so both qkv matmul operands have the contraction on
   partitions; qkT is produced directly in [qkv_col, token] (transposed) layout.
 - v is produced transposed too, then PE-transposed to token-major and
   augmented with a ones column (row-sum trick for the softmax denominator).
 - attention runs in the S^T = (K Q^T) layout, chunk-major: for each output
   token chunk of 512, for each head, accumulate P^T V into a [65, 512] PSUM
   tile (row 64 = softmax denominator via the ones column), then stream the
   denominator-reciprocal, normalization, projection, and output DMA for that
   chunk while later chunks compute. Causal structure skips invisible j-tiles;
   one triangular mask multiply per diagonal block.
 - PE-program-order stalls are avoided by deferring every PE-touching epilogue
   (reciprocal broadcast matmul, normalize, projection) two units behind the
   attention loop that produces its inputs.

All matmuls run in float32r (TF32-like single-pass fp32: ~1e-4 rel err,
4x faster than strict fp32 on the PE).
"""
import sys

sys.path.insert(0, "/opt/trn_rl_repo")

import numpy as np

B = 2
T = 2048
EMB = 1024
HEADS = 16
HD = EMB // HEADS  # 64
NCORES = 8
GROUPS = 4                 # head groups (cores per batch)
HPC = HEADS // GROUPS      # 4 heads per core
CQ = HPC * HD              # 256 q (or k or v) columns per core
NCB = 3 * CQ // 128        # 6 col-tiles of 128 in the qkv projection
KT = EMB // 128            # 8 contraction tiles
TCH = 512                  # token chunk
NCH = T // TCH             # 4 chunks
NTT = T // 128             # 16 token tiles
NR = CQ // 128             # 2 head-dim row tiles
SCALE = HD ** -0.5

_compiled = {}
ABLATE = None  # None | 's1' (stage1 only) | 's12' (no projection)


def _build(loop=1):
    import concourse.bass as bass
    import concourse.tile as tile
    from concourse import bacc, mybir
    from concourse.masks import make_identity

    F32 = mybir.dt.float32
    F16 = mybir.dt.float16
    AF = mybir.ActivationFunctionType

    nc = bacc.Bacc(None, target_bir_lowering=False)
    xT = nc.dram_tensor("xT", [EMB, T], F16, kind="ExternalInput")
    wqkv = nc.dram_tensor("wqkv", [EMB, 3 * CQ], F16, kind="ExternalInput")
    bqkv = nc.dram_tensor("bqkv", [128, NCB], F32, kind="ExternalInput")
    wproj = nc.dram_tensor("wproj", [CQ, EMB], F16, kind="ExternalInput")
    out = nc.dram_tensor("out", [T, EMB], F32, kind="ExternalOutput")

    xT_r = xT.rearrange("(kt p) t -> p kt t", p=128)
    wqkv_r = wqkv.rearrange("(kt p) c -> p kt c", p=128)
    wproj_r = wproj.rearrange("(r p) e -> p r e", p=128)

    with tile.TileContext(nc) as tc:
        with (
            tc.tile_pool(name="const", bufs=1) as const,
            tc.tile_pool(name="qk", bufs=1) as qkp,
            tc.tile_pool(name="xt", bufs=3) as xtp,
            tc.tile_pool(name="vt", bufs=2) as vtp,
            tc.tile_pool(name="pt", bufs=6) as ptp,
            tc.tile_pool(name="oh", bufs=1) as ohp,
            tc.tile_pool(name="den", bufs=5) as denp,
            tc.tile_pool(name="osb", bufs=3) as osbp,
            tc.tile_pool(name="psA", bufs=4, space="PSUM") as psA,
            tc.tile_pool(name="psO", bufs=4, space="PSUM") as psO,
        ):
            # ---- constants ----
            # weights on the scalar HWDGE queue, per k-tile, so the sync
            # queue's xt chunk loads run in parallel and matmuls start early
            bias_sb = const.tile([128, NCB], F32)
            nc.scalar.dma_start(out=bias_sb, in_=bqkv[:, :])
            w_sb = const.tile([128, KT, 3 * CQ], F16)
            for kt in range(KT):
                nc.scalar.dma_start(
                    out=w_sb[:, kt, 0:CQ], in_=wqkv_r[:, kt, 0:CQ]
                )
            for cp in range(1, 3):
                nc.scalar.dma_start(
                    out=w_sb[:, :, cp * CQ : (cp + 1) * CQ],
                    in_=wqkv_r[:, :, cp * CQ : (cp + 1) * CQ],
                )
            # stage-3 weights loaded inside body() after the xt chunks
            wp_sb = const.tile([128, NR, EMB], F16)
            ident = const.tile([128, 128], F16)
            make_identity(nc, ident)
            tri_f = const.tile([128, 128], F32)
            nc.gpsimd.memset(tri_f, 1.0)
            # keep where i(free) >= j(partition): -j + i >= 0
            nc.gpsimd.affine_select(
                out=tri_f, in_=tri_f,
                compare_op=mybir.AluOpType.is_ge,
                fill=0.0, base=0,
                pattern=[[1, 128]], channel_multiplier=-1,
            )
            tri = const.tile([128, 128], F16)
            nc.vector.tensor_copy(tri, tri_f)
            ones_f = const.tile([128, 64], F32)
            nc.vector.memset(ones_f, 1.0)
            ones64 = const.tile([1, 64], F16)
            nc.vector.tensor_copy(ones64, ones_f[0:1, :])
            # v in token-major, per (token_tile, head): 64 cols + ones col
            v_sb = const.tile([128, NTT, HPC, HD + 1], F16)
            nc.vector.tensor_copy(
                out=v_sb[:, :, :, HD : HD + 1],
                in_=ones_f.rearrange("p (a b c) -> p a b c", a=NTT, b=HPC),
            )
            qkT_sb = qkp.tile([128, 4, T], F16)
            ohT = ohp.tile([128, NR, T], F16)

            def body():
                # ---- stage 1: qkv projection (transposed layout) ----
                # qkT_sb[:, cb, t]: cb 0,1 = q col-tiles, 2,3 = k col-tiles
                def stage1_chunk(ch):
                    xt = xtp.tile([128, KT, TCH], F16)
                    if ch == 0:
                        for kt in range(KT):
                            nc.sync.dma_start(
                                out=xt[:, kt, :],
                                in_=xT_r[:, kt, ch * TCH : (ch + 1) * TCH],
                            )
                    else:
                        nc.sync.dma_start(
                            out=xt, in_=xT_r[:, :, ch * TCH : (ch + 1) * TCH]
                        )
                    ncb_run = 4 if ABLATE == "s1qk" else NCB
                    for cb in range(ncb_run):
                        ps = psA.tile([128, TCH], mybir.dt.float32, tag="psA")
                        for kt in range(KT):
                            nc.tensor.matmul(
                                ps,
                                w_sb[:, kt, cb * 128 : (cb + 1) * 128],
                                xt[:, kt, :],
                                start=(kt == 0),
                                stop=(kt == KT - 1),
                            )
                        if cb < 4:
                            nc.vector.tensor_scalar_add(
                                qkT_sb[:, cb, ch * TCH : (ch + 1) * TCH],
                                ps,
                                bias_sb[:, cb : cb + 1],
                            )
                        else:
                            vt = vtp.tile([128, TCH], F16)
                            nc.vector.tensor_scalar_add(
                                vt, ps, bias_sb[:, cb : cb + 1]
                            )
                            for s in range(TCH // 128 if ABLATE != "s1novt" else 0):
                                tt = ch * (TCH // 128) + s
                                tp = psA.tile([128, TCH], F16, tag="psA")
                                nc.tensor.transpose(
                                    tp[:, 0:128], vt[:, s * 128 : (s + 1) * 128], ident
                                )
                                for hh in range(2):
                                    h = 2 * (cb - 4) + hh
                                    nc.vector.tensor_copy(
                                        v_sb[:, tt, h, 0:HD],
                                        tp[:, hh * HD : (hh + 1) * HD],
                                    )

                # ---- stages 2+3: chunk-major attention + streaming epilogue ----
                pending = []  # (unit_idx, emit_fn): PE-touching epilogues, lag 2

                def flush(upto):
                    while pending and pending[0][0] <= upto:
                        pending.pop(0)[1]()

                def make_partB(psC2, rec_rows, r, base):
                    def partB():
                        # packed reciprocal broadcast: head-even -> psum rows
                        # 0:64 (col group 0), head-odd -> rows 64:128 (col
                        # group 64); they run concurrently on the PE
                        rp = psA.tile([128, TCH], mybir.dt.float32, tag="psA")
                        nc.tensor.matmul(
                            rp[0:64, :], ones64, rec_rows[0], start=True, stop=True
                        )
                        nc.tensor.matmul(
                            rp[64:128, :], ones64, rec_rows[1], start=True, stop=True
                        )
                        rec_sb = denp.tile([128, TCH], F32, tag="rec_sb")
                        nc.vector.tensor_copy(rec_sb, rp)
                        nc.vector.tensor_mul(
                            ohT[0:64, r, base : base + TCH],
                            psC2[0][0:64, :],
                            rec_sb[0:64, :],
                        )
                        nc.vector.tensor_mul(
                            ohT[64:128, r, base : base + TCH],
                            psC2[1][0:64, :],
                            rec_sb[64:128, :],
                        )
                    return partB

                def make_proj(cc):
                    def proj():
                        for tt in range(4 * cc, 4 * cc + 4):
                            for nn in range(EMB // 512):
                                pp = psA.tile(
                                    [128, TCH], mybir.dt.float32, tag="psA"
                                )
                                for r2 in range(NR):
                                    nc.tensor.matmul(
                                        pp,
                                        ohT[:, r2, tt * 128 : (tt + 1) * 128],
                                        wp_sb[:, r2, nn * 512 : (nn + 1) * 512],
                                        start=(r2 == 0),
                                        stop=(r2 == NR - 1),
                                    )
                                osb = osbp.tile([128, 512], F32)
                                nc.any.tensor_copy(osb, pp)
                                nc.sync.dma_start(
                                    out=out[
                                        tt * 128 : (tt + 1) * 128,
                                        nn * 512 : (nn + 1) * 512,
                                    ],
                                    in_=osb,
                                )
                    return proj

                unit = 0

                def emit_unit(cc, r):
                    # one unit = head pair (2r, 2r+1): their S^T matmuls use
                    # disjoint PE row groups (partition halves) and overlap
                    nonlocal unit
                    base = cc * TCH
                    jmax = 4 * cc + 3
                    psC2 = []
                    for _h in range(2):
                        psC_t = psO.tile([65, TCH], mybir.dt.float32, tag="psO")
                        psC2.append(psC_t)
                    diag = [j for j in range(4 * cc, jmax + 1) if j != 0]
                    rest = [j for j in range(1, 4 * cc)]
                    order = [0] + diag + rest
                    flush_pos = min(3, len(order) - 1)
                    prev = None  # PV lags S/exp by one j-tile
                    for pos, jt in enumerate(order):
                        i0 = 128 * jt
                        lo = max(base, i0)
                        hi = base + TCH
                        w = hi - lo
                        pts = []
                        for hh in range(2):
                            po = 64 * hh
                            sp = psA.tile([128, TCH], mybir.dt.float32, tag="psA")
                            nc.tensor.matmul(
                                sp[:, 0:w],
                                qkT_sb[po : po + 64, 2 + r, i0 : i0 + 128],
                                qkT_sb[po : po + 64, r, lo:hi],
                                start=True,
                                stop=True,
                            )
                            pt = ptp.tile([128, TCH], F16)
                            nc.scalar.activation(pt[:, 0:w], sp[:, 0:w], AF.Exp)
                            if i0 >= base:  # diagonal block: causal mask
                                nc.vector.tensor_mul(
                                    pt[:, 0:128], pt[:, 0:128], tri
                                )
                            pts.append(pt)
                        if pos == flush_pos:
                            flush(unit - 1)
                        if prev is not None:
                            _emit_pv_pair(nc, psC2, v_sb, prev, r, base, order[-1])
                        prev = (jt, pts, lo, hi)
                    _emit_pv_pair(nc, psC2, v_sb, prev, r, base, order[-1])

                    # part A: denominator chains (no PE instructions)
                    rec_rows = []
                    for hh in range(2):
                        den_row = denp.tile([1, TCH], F32, tag="den_row")
                        nc.vector.tensor_copy(den_row, psC2[hh][64:65, :])
                        den128 = denp.tile([128, TCH // 128], F32, tag="den128")
                        nc.scalar.dma_start(out=den128, in_=den_row)
                        rec128 = denp.tile([128, TCH // 128], F32, tag="rec128")
                        nc.vector.reciprocal(rec128, den128)
                        rec16 = denp.tile([128, TCH // 128], F16, tag="rec16")
                        nc.vector.tensor_copy(rec16, rec128)
                        rec_row = denp.tile([1, TCH], F16, tag="rec_row")
                        nc.scalar.dma_start(out=rec_row, in_=rec16)
                        rec_rows.append(rec_row)
                    pending.append((unit, make_partB(psC2, rec_rows, r, base)))
                    if r == NR - 1 and ABLATE != "s12":
                        pending.append((unit, make_proj(cc)))
                    unit += 1

                # interleave: attention units for chunk cc are emitted as soon
                # as stage-1 chunks 0..cc exist, so PE never starves on either
                # the stage-1 DMA feed or the attention epilogue latency
                stage1_chunk(0)
                stage1_chunk(1)
                if ABLATE == "s1":
                    stage1_chunk(2)
                    stage1_chunk(3)
                    return
                for r in range(NR):
                    emit_unit(0, r)
                # stage-3 weights: on the sync queue behind xt0/xt1, ready
                # well before proj(0) is flushed (re-loaded per loop iter)
                nc.sync.dma_start(out=wp_sb, in_=wproj_r)
                stage1_chunk(2)
                for r in range(NR):
                    emit_unit(1, r)
                stage1_chunk(3)
                for r in range(NR):
                    emit_unit(2, r)
                for r in range(NR):
                    emit_unit(3, r)
                flush(unit)

            if loop == 1:
                body()
            else:
                with tc.For_i(
                    0, loop, 1,
                    hint_engines=(
                        mybir.EngineType.PE,
                        mybir.EngineType.Activation,
                        mybir.EngineType.DVE,
                        mybir.EngineType.SP,
                        mybir.EngineType.Pool,
                    ),
                ):
                    body()

    nc.finalize()
    return nc


def _emit_pv_pair(nc, psC2, v_sb, prev, r, base, jlast):
    """P^T[jt] @ v_aug for both heads of the pair, accumulated into their
    chunk PSUM tiles."""
    jt, pts, lo, hi = prev
    for hh in range(2):
        nc.tensor.matmul(
            psC2[hh][:, lo - base : hi - base],
            v_sb[:, jt, 2 * r + hh, :],
            pts[hh][:, 0 : hi - lo],
            start=(jt == 0),
            stop=(jt == jlast),
            skip_group_check=(jt != 0),
        )


def _shard_inputs(x, Wqkv, bqkv, Wproj):
    """Build the 8 per-core input maps."""
    x = np.asarray(x, dtype=np.float32)
    Wqkv = np.asarray(Wqkv, dtype=np.float32)
    bqkv = np.asarray(bqkv, dtype=np.float32)
    Wproj = np.asarray(Wproj, dtype=np.float32)

    in_maps = []
    for c in range(NCORES):
        b = c // GROUPS
        g = c % GROUPS
        cols = slice(g * CQ, (g + 1) * CQ)
        wq = Wqkv[:, cols] * SCALE
        wk = Wqkv[:, EMB:][:, cols]
        wv = Wqkv[:, 2 * EMB:][:, cols]
        w_c = np.ascontiguousarray(
            np.concatenate([wq, wk, wv], axis=1).astype(np.float16)
        )
        bq = bqkv[cols] * SCALE
        bk = bqkv[EMB:][cols]
        bv = bqkv[2 * EMB:][cols]
        b_c = np.concatenate([bq, bk, bv])  # [768]
        b_c = np.ascontiguousarray(b_c.reshape(NCB, 128).T)  # [128, 6]
        wp_c = np.ascontiguousarray(Wproj[cols, :].astype(np.float16))
        xT_c = np.ascontiguousarray(x[b].T.astype(np.float16))  # [1024, 2048]
        in_maps.append({"xT": xT_c, "wqkv": w_c, "bqkv": b_c, "wproj": wp_c})
    return in_maps


def run(inputs, trace=False, **kwargs):
    """Build (cached), run on 8 cores, return (full_output, BassKernelResults)."""
    from concourse.bass_utils import run_bass_kernel_spmd

    if _compiled.get(1) is None:
        _compiled[1] = _build()
    in_maps = _shard_inputs(
        inputs["x"], inputs["Wqkv"], inputs["bqkv"], inputs["Wproj"]
    )
    res = run_bass_kernel_spmd(
        _compiled[1], in_maps, core_ids=list(range(NCORES)), trace=trace, **kwargs
    )
    partials = np.stack([res.results[c]["out"] for c in range(NCORES)])  # [8,T,EMB]
    bproj = np.asarray(inputs["bproj"], dtype=np.float32)
    full = np.stack(
        [partials[b * GROUPS : (b + 1) * GROUPS].sum(axis=0) for b in range(B)]
    ) + bproj
    return full.astype(np.float32), res


def kernel(**inputs):
    out, _ = run(inputs)
    return out

